# revision 2
# baseline (speedup 1.0000x reference)
"""CARAFE-3D (scale=2, k_up=5) Trainium2 kernel, v3.

Parity sharding (8 cores = 8 pixel-shuffle parities), coarse 32^3 grid.
Per output voxel: out[c] = sum_t softmax_t(L) * X[c, +delta_t], 125 taps.

v3 reassembly (per 1-h-row tile, F=1024 = 32w x 32d voxels):
  - conv2 as 15 pair-packed MMs per 512-half (9 z-pairs K=128, 3 w-pairs
    K=128 via a +1w shifted copy h2w, 3 singles K=64).
  - E = exp(logits+b2) in a quadrant-strided tap layout: tap at partition
    32*q + lane so a single DVE stream_shuffle (mask=[lane]*32) broadcasts
    4 taps (one per quadrant) to 128 partitions at copy speed.
  - per 4-tap group: rep via DVE stream_shuffle (path A/E) or PE sel-MM to
    PSUM + ACT copy to bf16 (path B); prod = rep*xv on DVE (bf16 2x) or
    POOL; collapse via PE eye-MM accumulation into col PSUM (some pairs
    pre-added on DVE).
  - denominator: all-ones sel MM -> S replicated on 32 rows; 1/S =
    exp(-ln S) on ACT; out = col * rinv on DVE -> DMA.
  - no barriers: tile framework schedules; reassembly software-pipelined
    one tile behind conv2.
"""
import sys
sys.path.insert(0, '/opt/trn_rl_repo')

import numpy as np
import ml_dtypes

# ---------------- geometry ----------------
B, CIN, MID, D = 2, 32, 64, 32
T = 125
NG = 32                     # 25 l-quads, 5 j-quads, 1 h-quad, 1 single
PD, PDW, PDD = 36, 40, 40   # padded h extent; padded w/d extents
W = 32                      # full-width tiles
N_TILE = W * D              # 1024
NH = 512                    # psum half
SLAB_H = 4
N_SLAB = D // SLAB_H
EPS = 1e-5

# ---------------- path plan (tunable) ----------------
# 'A': shuffle + DVE mul     'E': shuffle + POOL mul
# 'B': sel-MM + ACT copy + DVE mul
PATHS = list('EBABEBABEBABEBABEBABEBABEBABEBAB')
PATHS[6] = 'E'
PATHS[22] = 'E'
assert len(PATHS) == NG
# pairs of consecutive groups whose prods are pre-added on DVE (each pair
# saves one collapse MM); must not include group 31 (32-row prod)
PREADD = [(2, 3), (10, 11), (18, 19), (26, 27)]

_cache = {}


# ---------------- tap -> partition map ----------------
def _tap_part():
    """(i,j,l) -> partition in the quadrant-strided E layout."""
    m = {}
    for i in range(5):
        for j in range(5):
            for l in range(4):
                m[(i, j, l)] = 32 * l + (5 * i + j)
    for i in range(5):
        for j in range(4):
            m[(i, j, 4)] = 32 * j + 25 + i
    for i in range(4):
        m[(i, 4, 4)] = 32 * i + 30
    m[(4, 4, 4)] = 31
    assert len(m) == 125
    parts = sorted(m.values())
    assert len(set(parts)) == 125
    return m


def _group_lane(g):
    """mask lane for group g (same lane in every quadrant)."""
    if g < 25:
        i, j = g // 5, g % 5
        return 5 * i + j
    if g < 30:
        return 25 + (g - 25)
    if g == 30:
        return 30
    return 31


def _build_host_constants(inputs):
    X = np.asarray(inputs['X'], np.float32)
    w_comp = np.asarray(inputs['w_comp'], np.float32)[:, :, 0, 0, 0]
    w_enc = np.asarray(inputs['w_enc'], np.float32)
    inv1 = np.asarray(inputs['gamma1'], np.float32) / np.sqrt(np.asarray(inputs['var1'], np.float32) + EPS)
    b1 = np.asarray(inputs['beta1'], np.float32) - np.asarray(inputs['mean1'], np.float32) * inv1
    inv2 = np.asarray(inputs['gamma2'], np.float32) / np.sqrt(np.asarray(inputs['var2'], np.float32) + EPS)
    b2 = np.asarray(inputs['beta2'], np.float32) - np.asarray(inputs['mean2'], np.float32) * inv2

    w1 = (w_comp * inv1[:, None])
    w1T = np.ascontiguousarray(w1.T)
    w2 = w_enc * inv2[:, None, None, None, None]     # (1000, 64, 3,3,3)

    bf = ml_dtypes.bfloat16
    tp = _tap_part()

    xpad = np.zeros((B, CIN, PD, PDW, PDD), np.float32)
    xpad[:, :, 2:34, 2:34, 2:34] = X
    xpad_bf = xpad.astype(bf)

    # collapse lhsT: [128, 32] eye blocks; single-group [32, 32] identity
    eye32 = np.zeros((128, 32), np.float32)
    for l in range(4):
        eye32[l * 32:(l + 1) * 32] = np.eye(32)
    e32 = np.eye(32, dtype=np.float32)
    # S lhsT: [128, 32] ones at used-tap partitions
    onesS = np.zeros((128, 32), np.float32)
    for p in tp.values():
        onesS[p, :] = 1.0
    # sel matrices for B-path groups: [128, 128], col op selects
    # partition 32*(op//32) + lane(g)
    selg = {}
    for g in range(NG):
        if PATHS[g] != 'B':
            continue
        m = _group_lane(g)
        sel = np.zeros((128, 128), np.float32)
        for q in range(4):
            sel[32 * q + m, 32 * q:32 * q + 32] = 1.0
        selg[g] = sel.astype(bf)
    sel_stack = (np.stack([selg[g] for g in sorted(selg)])
                 if selg else np.zeros((1, 128, 128), np.float32).astype(bf))

    per_core = []
    for off in range(8):
        # tap (i,j,l) -> original channel ((i*25+j*5+l)*8 + off)
        w2part = np.zeros((128, 64, 3, 3, 3), np.float32)
        b2part = np.zeros((128, 1), np.float32)
        for (i, j, l), p in tp.items():
            ch = (i * 25 + j * 5 + l) * 8 + off
            w2part[p] = w2[ch]
            b2part[p, 0] = b2[ch]
        # z-pairs: (di,dj) with dl=0 over dl=2
        w2p = np.zeros((9, 128, 128), np.float32)
        for di in range(3):
            for dj in range(3):
                w2p[di * 3 + dj, :MID, :] = w2part[:, :, di, dj, 0].T
                w2p[di * 3 + dj, MID:, :] = w2part[:, :, di, dj, 2].T
        # w-pairs: (di, dj=0 over dj=1, dl=1)
        w2w = np.zeros((3, 128, 128), np.float32)
        for di in range(3):
            w2w[di, :MID, :] = w2part[:, :, di, 0, 1].T
            w2w[di, MID:, :] = w2part[:, :, di, 1, 1].T
        # singles: (di, dj=2, dl=1)
        w2s = np.zeros((3, MID, 128), np.float32)
        for di in range(3):
            w2s[di] = w2part[:, :, di, 2, 1].T
        per_core.append({
            'w2p': w2p.astype(bf),
            'w2w': w2w.astype(bf),
            'w2s': w2s.astype(bf),
            'b2': b2part,
        })

    shared = {
        'xpad': xpad_bf,
        'w1T': w1T.astype(bf),
        'b1': b1.reshape(MID, 1).astype(np.float32),
        'eye32': eye32.astype(bf),
        'e32': e32.astype(bf),
        'onesS': onesS.astype(bf),
        'selg': sel_stack,
    }
    return shared, per_core


def _in_maps(shared, per_core):
    maps = []
    for off in range(8):
        m = dict(shared)
        m.update(per_core[off])
        maps.append(m)
    return maps


# ---------------- bass program ----------------
def _build_nc(n_slabs=N_SLAB, n_batches=B, for_hw=False):
    import concourse.bass as bass
    import concourse.bacc as bacc
    import concourse.mybir as mybir
    import concourse.tile as tile

    F32 = mybir.dt.float32
    BF16 = mybir.dt.bfloat16
    AF = mybir.ActivationFunctionType

    n_selg = sum(1 for p in PATHS if p == 'B')
    sel_idx = {}
    for g in range(NG):
        if PATHS[g] == 'B':
            sel_idx[g] = len(sel_idx)

    nc = bacc.Bacc() if for_hw else bass.Bass(target_bir_lowering=False)

    xpad_d = nc.declare_dram_parameter("xpad", [B, CIN, PD, PDW, PDD], BF16, isOutput=False)
    w1_d = nc.declare_dram_parameter("w1T", [CIN, MID], BF16, isOutput=False)
    b1_d = nc.declare_dram_parameter("b1", [MID, 1], F32, isOutput=False)
    w2p_d = nc.declare_dram_parameter("w2p", [9, 128, 128], BF16, isOutput=False)
    w2w_d = nc.declare_dram_parameter("w2w", [3, 128, 128], BF16, isOutput=False)
    w2s_d = nc.declare_dram_parameter("w2s", [3, MID, 128], BF16, isOutput=False)
    b2_d = nc.declare_dram_parameter("b2", [128, 1], F32, isOutput=False)
    eye32_d = nc.declare_dram_parameter("eye32", [128, 32], BF16, isOutput=False)
    e32_d = nc.declare_dram_parameter("e32", [32, 32], BF16, isOutput=False)
    onesS_d = nc.declare_dram_parameter("onesS", [128, 32], BF16, isOutput=False)
    selg_d = nc.declare_dram_parameter("selg", [max(n_selg, 1), 128, 128], BF16, isOutput=False)
    out_d = nc.declare_dram_parameter("out", [B, CIN, D, D, D], F32, isOutput=True)
    import os as _os
    DBG = _os.environ.get('K3DBG', '0') == '1'
    if DBG:
        dbg_e_d = nc.declare_dram_parameter("dbg_e", [128, N_TILE], BF16, isOutput=True)
        dbg_h_d = nc.declare_dram_parameter("dbg_h", [128, 6, 34, 34], BF16, isOutput=True)
        dbg_w_d = nc.declare_dram_parameter("dbg_w", [128, 6, 33, 32], BF16, isOutput=True)
        dbg_done = [False]

    with tile.TileContext(nc) as tc:
        with tc.tile_pool(name="consts", bufs=1) as consts, \
             tc.tile_pool(name="x4z", bufs=2) as x4z_pool, \
             tc.tile_pool(name="x4w", bufs=2) as x4w_pool, \
             tc.tile_pool(name="x4h", bufs=2) as x4h_pool, \
             tc.tile_pool(name="h2", bufs=1) as h2_pool, \
             tc.tile_pool(name="ebf", bufs=2) as ebf_pool, \
             tc.tile_pool(name="rb", bufs=3) as rb_pool, \
             tc.tile_pool(name="prodb", bufs=6) as prodb_pool, \
             tc.tile_pool(name="epb", bufs=8) as epb_pool, \
             tc.tile_pool(name="lns", bufs=2) as lns_pool, \
             tc.tile_pool(name="rinv", bufs=2) as rinv_pool, \
             tc.tile_pool(name="outt", bufs=2) as outt_pool, \
             tc.tile_pool(name="xt", bufs=2) as xt_pool, \
             tc.tile_pool(name="psum_c2", bufs=1, space="PSUM") as ps_c2, \
             tc.tile_pool(name="psum_rep", bufs=2, space="PSUM") as ps_rep, \
             tc.tile_pool(name="psum_col", bufs=1, space="PSUM") as ps_col:

            # ---- constants ----
            w1_t = consts.tile([CIN, MID], BF16, tag="w1")
            nc.sync.dma_start(out=w1_t, in_=w1_d[:, :])
            b1_t = consts.tile([MID, 1], F32, tag="b1")
            nc.sync.dma_start(out=b1_t, in_=b1_d[:, :])
            w2p_t = consts.tile([128, 9, 128], BF16, tag="w2p")
            nc.sync.dma_start(out=w2p_t, in_=w2p_d.ap().transpose([1, 0, 2]))
            w2w_t = consts.tile([128, 3, 128], BF16, tag="w2w")
            nc.sync.dma_start(out=w2w_t, in_=w2w_d.ap().transpose([1, 0, 2]))
            w2s_t = consts.tile([MID, 3, 128], BF16, tag="w2s")
            nc.sync.dma_start(out=w2s_t, in_=w2s_d.ap().transpose([1, 0, 2]))
            b2_t = consts.tile([128, 1], F32, tag="b2")
            nc.sync.dma_start(out=b2_t, in_=b2_d[:, :])
            eye32_t = consts.tile([128, 32], BF16, tag="eye32")
            nc.sync.dma_start(out=eye32_t, in_=eye32_d[:, :])
            e32_t = consts.tile([32, 32], BF16, tag="e32")
            nc.sync.dma_start(out=e32_t, in_=e32_d[:, :])
            onesS_t = consts.tile([128, 32], BF16, tag="onesS")
            nc.sync.dma_start(out=onesS_t, in_=onesS_d[:, :])
            sel_t = None
            if n_selg:
                sel_t = consts.tile([128, n_selg, 128], BF16, tag="selg")
                nc.sync.dma_start(out=sel_t, in_=selg_d.ap().transpose([1, 0, 2]))

            tiles_list = []
            for b in range(n_batches):
                for sl in range(n_slabs):
                    for hh in range(SLAB_H):
                        tiles_list.append((b, sl, hh))
            n_tiles = len(tiles_list)

            slab_tiles = {}
            first_h2 = [True]

            def emit_slab_prep(b, sl):
                h0 = sl * SLAB_H
                x4z = x4z_pool.tile([128, 8, 36, 32], BF16, tag="x4z")
                for ll in range(4):
                    for hr in range(8):
                        nc.sync.dma_start(
                            out=x4z[ll * 32:(ll + 1) * 32, hr, :, :],
                            in_=xpad_d[b, :, h0 + hr, 0:36, ll:ll + 32])
                x4w = x4w_pool.tile([128, 8, 32, 32], BF16, tag="x4w")
                for jj in range(4):
                    for hr in range(8):
                        nc.sync.dma_start(
                            out=x4w[jj * 32:(jj + 1) * 32, hr, :, :],
                            in_=xpad_d[b, :, h0 + hr, jj:jj + 32, 4:36])
                x4h = x4h_pool.tile([128, 8, 32, 32], BF16, tag="x4h")
                for ii in range(4):
                    hr_max = 8 if ii == 0 else SLAB_H
                    for hr in range(hr_max):
                        nc.sync.dma_start(
                            out=x4h[ii * 32:(ii + 1) * 32, hr, :, :],
                            in_=xpad_d[b, :, h0 + hr + ii, 4:36, 4:36])

                # conv1 -> h2z (mid + mid shifted +2d); h2w (mid@d+1, +1w)
                h2z = h2_pool.tile([128, 6, 34, 34], BF16, tag="h2z")
                h2w = h2_pool.tile([128, 6, 33, 32], BF16, tag="h2w")
                if first_h2[0]:
                    nc.vector.memset(h2z, 0.0)
                    first_h2[0] = False
                else:
                    for phr in range(6):
                        h_real = h0 + phr - 1
                        if h_real < 0 or h_real >= D:
                            nc.vector.memset(h2z[:, phr], 0.0)
                for phr in range(6):
                    h_real = h0 + phr - 1
                    if h_real < 0 or h_real >= D:
                        continue
                    xt = xt_pool.tile([CIN, N_TILE], BF16, tag="xt")
                    nc.sync.dma_start(
                        out=xt, in_=xpad_d[b, :, h_real + 2, 2:34, 2:34])
                    c1 = ps_rep.tile([MID, N_TILE], F32, tag="rep")
                    nc.tensor.matmul(c1[:, 0:NH], w1_t, xt[:, 0:NH],
                                     start=True, stop=True,
                                     skip_group_check=True)
                    nc.tensor.matmul(c1[:, NH:], w1_t, xt[:, NH:],
                                     start=True, stop=True,
                                     skip_group_check=True)
                    dst = h2z[0:MID, phr, 1:33, 1:33]
                    if phr % 2 == 0:
                        nc.scalar.activation(out=dst, in_=c1, func=AF.Relu,
                                             bias=b1_t, scale=1.0)
                    else:
                        nc.vector.tensor_scalar(
                            out=dst, in0=c1, scalar1=b1_t, scalar2=0.0,
                            op0=mybir.AluOpType.add, op1=mybir.AluOpType.max)
                for phr in range(6):
                    h_real = h0 + phr - 1
                    if h_real < 0 or h_real >= D:
                        nc.vector.memset(h2w[:, phr], 0.0)
                        continue
                    nc.sync.dma_start(
                        out=h2z[MID:128, phr, :, 0:32],
                        in_=h2z[0:MID, phr, :, 2:34])
                    nc.sync.dma_start(
                        out=h2w[0:MID, phr, :, :],
                        in_=h2z[0:MID, phr, 0:33, 1:33])
                    nc.sync.dma_start(
                        out=h2w[MID:128, phr, :, :],
                        in_=h2z[0:MID, phr, 1:34, 1:33])
                slab_tiles[(b, sl)] = dict(x4z=x4z, x4w=x4w, x4h=x4h,
                                           h2z=h2z, h2w=h2w)

            def make_conv2(st, hh):
                """conv2 + exp as a list of thunks to interleave into the
                previous tile's PE stream; returns (e_bf, thunks)."""
                h2z, h2w = st['h2z'], st['h2w']
                c2 = ps_c2.tile([128, N_TILE], F32, tag="c2")
                e_bf = ebf_pool.tile([128, N_TILE], BF16, tag="ebf")
                thunks = []
                for half in range(2):
                    w0 = 16 * half
                    c2h = c2[:, half * NH:(half + 1) * NH]
                    mms = []
                    for di in range(3):
                        for dj in range(3):
                            mms.append((w2p_t[:, di * 3 + dj, :],
                                        h2z[:, hh + di,
                                            dj + w0:dj + w0 + 16, 0:32]))
                    for di in range(3):
                        mms.append((w2w_t[:, di, :],
                                    h2w[:, hh + di, w0:w0 + 16, 0:32]))
                    for di in range(3):
                        mms.append((w2s_t[:, di, :],
                                    h2z[0:MID, hh + di, 2 + w0:18 + w0,
                                        1:33]))
                    for k, (lhs, rhs) in enumerate(mms):
                        def th(lhs=lhs, rhs=rhs, c2h=c2h, k=k):
                            nc.tensor.matmul(c2h, lhs, rhs, start=(k == 0),
                                             stop=(k == 14),
                                             skip_group_check=True)
                        thunks.append(th)

                    def th_exp(c2h=c2h, half=half):
                        nc.scalar.activation(
                            out=e_bf[:, half * NH:(half + 1) * NH], in_=c2h,
                            func=AF.Exp, bias=b2_t, scale=1.0)
                    thunks.append(th_exp)
                return e_bf, thunks

            def emit_denom(e_bf):
                """S (replicated on 32 rows) -> rinv32 = 1/S in bf16."""
                s32 = ps_rep.tile([32, N_TILE], F32, tag="rep")
                nc.tensor.matmul(s32[:, 0:NH], onesS_t, e_bf[:, 0:NH],
                                 start=True, stop=True, skip_group_check=True)
                nc.tensor.matmul(s32[:, NH:], onesS_t, e_bf[:, NH:],
                                 start=True, stop=True, skip_group_check=True)
                lns = lns_pool.tile([32, N_TILE], F32, tag="lns")
                nc.scalar.activation(out=lns, in_=s32, func=AF.Ln, scale=1.0)
                rinv32 = rinv_pool.tile([32, N_TILE], BF16, tag="rinv")
                nc.scalar.activation(out=rinv32, in_=lns, func=AF.Exp,
                                     scale=-1.0)
                return rinv32

            def xview(st, g, hh):
                if g < 25:
                    i, j = g // 5, g % 5
                    return st['x4z'][:, hh + i, j:j + 32, 0:32]
                if g < 30:
                    i = g - 25
                    return st['x4w'][:, hh + i, :, :]
                if g == 30:
                    return st['x4h'][:, hh, :, :]
                return st['x4h'][0:32, hh + 4, :, :]

            def emit_reassembly(st, hh, e_bf, rinv32, b, h, feed=()):
                feed = list(feed)

                def step(n=1):
                    for _ in range(n):
                        if feed:
                            feed.pop(0)()

                col = ps_col.tile([32, N_TILE], F32, tag="col")
                col_started = [False, False]
                pend_pre = {}   # g -> prod tile awaiting its preadd partner
                pre_partner = {}
                for ga, gb in PREADD:
                    pre_partner[ga] = gb
                    pre_partner[gb] = ga

                def collapse(prod, rows=128, last=False):
                    lhs = eye32_t if rows == 128 else e32_t
                    for half in range(2):
                        nc.tensor.matmul(
                            col[:, half * NH:(half + 1) * NH],
                            lhs[0:rows, :],
                            prod[0:rows, half * NH:(half + 1) * NH],
                            start=(not col_started[half]), stop=last,
                            skip_group_check=True)
                        col_started[half] = True

                def emit_mul(g):
                    path = PATHS[g]
                    rows = 32 if g == 31 else 128
                    lane = _group_lane(g)
                    xv = xview(st, g, hh)
                    pool = epb_pool if path == 'E' else prodb_pool
                    prod = pool.tile([128, N_TILE], BF16,
                                     tag="epb" if path == 'E' else "pb")
                    if path == 'B':
                        rep = ps_rep.tile([128, N_TILE], F32, tag="rep")
                        nc.tensor.matmul(rep[:, 0:NH],
                                         sel_t[:, sel_idx[g], :],
                                         e_bf[:, 0:NH], start=True, stop=True,
                                         skip_group_check=True)
                        nc.tensor.matmul(rep[:, NH:], sel_t[:, sel_idx[g], :],
                                         e_bf[:, NH:], start=True, stop=True,
                                         skip_group_check=True)
                        rb = rb_pool.tile([128, N_TILE], BF16, tag="rb")
                        nc.scalar.activation(out=rb[0:rows], in_=rep[0:rows],
                                             func=AF.Copy, scale=1.0)
                        nc.vector.tensor_mul(out=prod[0:rows], in0=xv,
                                             in1=rb[0:rows])
                    else:
                        rb = rb_pool.tile([128, N_TILE], BF16, tag="rb")
                        nc.vector.stream_shuffle(out=rb[0:rows],
                                                 in_=e_bf[0:rows],
                                                 mask=[lane] * 32)
                        if path == 'A':
                            nc.vector.tensor_mul(out=prod[0:rows], in0=xv,
                                                 in1=rb[0:rows])
                        else:
                            nc.gpsimd.tensor_mul(out=prod[0:rows], in0=xv,
                                                 in1=rb[0:rows])
                    return prod, rows

                # A/B groups drive the loop; POOL-path (E) muls fed in early
                # and steadily so the pool queue never starves, their
                # collapses deferred so the in-order PE never waits on pool
                LAG = 3
                queue = []
                ab = [g for g in range(NG) if PATHS[g] != 'E']
                egs = [g for g in range(NG) if PATHS[g] == 'E']
                eprods = []
                ei = 0
                for _ in range(2):
                    if eprods is not None and len(eprods) < len(egs):
                        eprods.append(emit_mul(egs[len(eprods)]))
                for j, g in enumerate(ab):
                    step(1)
                    if j % 3 == 0 and len(eprods) < len(egs):
                        eprods.append(emit_mul(egs[len(eprods)]))
                    prod, rows = emit_mul(g)
                    step(1)
                    partner = pre_partner.get(g)
                    if partner is None:
                        queue.append((prod, rows))
                    elif partner in pend_pre:
                        psum_b = pend_pre.pop(partner)
                        pre = prodb_pool.tile([128, N_TILE], BF16, tag="pb")
                        nc.vector.tensor_add(out=pre, in0=psum_b, in1=prod)
                        queue.append((pre, 128))
                    else:
                        pend_pre[g] = prod
                    while len(queue) > LAG:
                        collapse(*queue.pop(0))
                    if j >= 8 and ei < len(eprods) and ei < (j - 8) // 2 + 1:
                        collapse(*eprods[ei])
                        ei += 1
                # phase 3: drain
                remaining = queue + eprods[ei:]
                for k, (prod, rows) in enumerate(remaining):
                    step(1)
                    collapse(prod, rows, last=(k == len(remaining) - 1))
                step(len(feed))

                outt = outt_pool.tile([32, N_TILE], F32, tag="outt")
                nc.vector.tensor_mul(out=outt, in0=col, in1=rinv32)
                nc.sync.dma_start(out=out_d[b, :, h, :, :], in_=outt)

            # ---------------- main loop (1-tile software pipeline) ----
            pend = None     # (st, hh, e_bf, rinv32, b, h) awaiting reassembly
            for t_idx, (b, sl, hh) in enumerate(tiles_list):
                if (b, sl) not in slab_tiles:
                    emit_slab_prep(b, sl)
                st = slab_tiles[(b, sl)]
                e_bf, thunks = make_conv2(st, hh)
                if DBG and not dbg_done[0]:
                    dbg_done[0] = True
                    for th in thunks:
                        th()
                    thunks = []
                    nc.sync.dma_start(out=dbg_e_d[:, :], in_=e_bf)
                    nc.sync.dma_start(out=dbg_h_d.ap(), in_=st['h2z'])
                    nc.sync.dma_start(out=dbg_w_d.ap(), in_=st['h2w'])
                if pend is not None:
                    emit_reassembly(*pend, feed=thunks)
                else:
                    for th in thunks:
                        th()
                rinv32 = emit_denom(e_bf)
                pend = (st, hh, e_bf, rinv32, b, sl * SLAB_H + hh)
            if pend is not None:
                emit_reassembly(*pend)

    if for_hw:
        nc.compile()
    return nc


def _get_nc(key, **kw):
    if key not in _cache:
        _cache[key] = _build_nc(**kw)
    return _cache[key]


# ---------------- host entry ----------------
def kernel(**inputs):
    from concourse.bass_utils import run_bass_kernel_spmd

    shared, per_core = _build_host_constants(inputs)
    nc = _get_nc('full', for_hw=True)

    res = run_bass_kernel_spmd(nc, _in_maps(shared, per_core), list(range(8)))
    out = np.zeros((B, CIN, 64, 64, 64), np.float32)
    for off in range(8):
        si, sj, slp = (off >> 2) & 1, (off >> 1) & 1, off & 1
        out[:, :, si::2, sj::2, slp::2] = np.asarray(
            res.results[off]['out']).reshape(B, CIN, D, D, D)
    return out
